# revision 16
# baseline (speedup 1.0000x reference)
"""DPFP delta-rule attention kernel for 8 trn2 NeuronCores.

Sharding: 32 (batch, head) units -> core c owns batch c//2, heads 4*(c%2)..+4.
Each core computes qkv for its head-group, dpfp features, the single global
delta-rule update of its W slice, readout, and a partial output projection
(its 256 rows of wout). Host sums the two partials per batch and adds bout.
"""

import sys

if "/opt/trn_rl_repo" not in sys.path:
    sys.path.insert(0, "/opt/trn_rl_repo")

import numpy as np
import ml_dtypes

BF16 = ml_dtypes.bfloat16

B, N, DIM = 4, 4096, 512
H, D, S = 8, 64, 128  # heads, dim_head, dpfp feature dim (2*nu*d)
HG = 4  # heads per core
P = 128
NCH = N // P  # 32 token chunks
SCALE = 1.0 / D**0.5

_CACHE = {}


def _build_nc():
    import concourse.bass as bass
    import concourse.tile as tile
    from concourse import bacc
    from concourse import mybir
    from concourse.masks import make_identity

    f32 = mybir.dt.float32
    bf16 = mybir.dt.bfloat16
    AF = mybir.ActivationFunctionType
    OP = mybir.AluOpType

    nc = bacc.Bacc()

    xt_d = nc.dram_tensor("xt", [DIM, N], bf16, kind="ExternalInput")
    wqkvb_d = nc.dram_tensor("wqkvb", [DIM, 772], bf16, kind="ExternalInput")
    wh1_d = nc.dram_tensor("wh1", [HG, S, 65], bf16, kind="ExternalInput")
    wf_d = nc.dram_tensor("wf", [HG, S, D], f32, kind="ExternalInput")
    wout_d = nc.dram_tensor("wout_s", [HG * D, DIM], bf16, kind="ExternalInput")
    bbrow_d = nc.dram_tensor("bbrow", [1, 260], bf16, kind="ExternalInput")

    out_d = nc.dram_tensor("out_part", [N, DIM], f32, kind="ExternalOutput")
    wnew_d = nc.dram_tensor("w_new", [HG, S, D], f32, kind="ExternalOutput")

    from contextlib import ExitStack

    with tile.TileContext(nc) as tc, ExitStack() as ctx:
        const = ctx.enter_context(tc.tile_pool(name="const", bufs=1))
        work = ctx.enter_context(tc.tile_pool(name="work", bufs=3))
        coef = ctx.enter_context(tc.tile_pool(name="coef", bufs=4))
        psA = ctx.enter_context(tc.tile_pool(name="psA", bufs=3, space="PSUM"))
        psT = ctx.enter_context(tc.tile_pool(name="psT", bufs=2, space="PSUM"))
        psB = ctx.enter_context(tc.tile_pool(name="psB", bufs=2, space="PSUM"))
        psW = ctx.enter_context(tc.tile_pool(name="psW", bufs=1, space="PSUM"))

        # ---- persistent SBUF tensors ----
        xt = const.tile([P, 4, N], bf16)
        nc.sync.dma_start(out=xt, in_=xt_d.rearrange("(ko p) n -> p ko n", p=P))
        wqkvb = const.tile([P, 4, 772], bf16)
        nc.sync.dma_start(
            out=wqkvb, in_=wqkvb_d.rearrange("(ko p) c -> p ko c", p=P)
        )
        wh1 = const.tile([P, HG, 65], bf16)
        nc.sync.dma_start(out=wh1, in_=wh1_d.rearrange("h p c -> p h c"))
        wf = const.tile([P, HG, D], f32)
        nc.sync.dma_start(out=wf, in_=wf_d.rearrange("h p d -> p h d"))
        wout = const.tile([P, 2, DIM], bf16)
        nc.sync.dma_start(
            out=wout, in_=wout_d.rearrange("(ko p) c -> p ko c", p=P)
        )
        bbrow = const.tile([1, 260], bf16)
        nc.sync.dma_start(out=bbrow, in_=bbrow_d[:, :])
        onesrow = const.tile([1, P], bf16)
        nc.vector.memset(onesrow, 1.0)

        ident = const.tile([P, P], bf16)
        make_identity(nc, ident)

        v_sb = const.tile([P, NCH, 256], bf16)  # v per chunk
        betab_sb = const.tile([P, NCH, HG], f32)  # beta + bbeta per chunk
        qT_sb = const.tile([P, NCH, HG * P], bf16)  # dpfp(q)^T, 4 heads

        pW = psW.tile([P, HG, D], f32)  # delta-rule accumulator

        # =============== phase 1: k-side + W update ===============
        for c in range(NCH):
            tok = slice(c * P, (c + 1) * P)
            pa = psA.tile([P, 512], f32, tag="big")
            pb = psB.tile([P, 260], f32, tag="small")
            for kc in range(4):
                nc.tensor.matmul(
                    pa, xt[:, kc, tok], wqkvb[:, kc, 0:512],
                    start=(kc == 0), stop=(kc == 3),
                )
            for kc in range(4):
                nc.tensor.matmul(
                    pb, xt[:, kc, tok], wqkvb[:, kc, 512:772],
                    start=(kc == 0), stop=False,
                )
            nc.tensor.matmul(pb, onesrow, bbrow, start=False, stop=True)
            qk_sb = work.tile([P, 512], bf16, tag="qk_sb")
            nc.vector.tensor_copy(qk_sb, pa)
            qmm = qk_sb[:, 0:256].rearrange("p (h d) -> p h d", h=HG)
            kmm = qk_sb[:, 256:512].rearrange("p (h d) -> p h d", h=HG)

            fq = work.tile([P, HG, S], bf16, tag="fq")
            fk = work.tile([P, HG, S], bf16, tag="fk")
            nc.scalar.activation(out=fq[:, :, 0:D], in_=qmm, func=AF.Relu, scale=1.0)
            nc.scalar.activation(out=fq[:, :, D:S], in_=qmm, func=AF.Relu, scale=-1.0)
            nc.scalar.activation(out=fk[:, :, 0:D], in_=kmm, func=AF.Relu, scale=1.0)
            nc.scalar.activation(out=fk[:, :, D:S], in_=kmm, func=AF.Relu, scale=-1.0)
            # single evacuation of pb (v | beta+bbeta), then split in SBUF
            vb = work.tile([P, 260], bf16, tag="vb")
            nc.scalar.copy(out=vb, in_=pb)
            nc.vector.tensor_copy(v_sb[:, c], vb[:, 0:256])
            nc.vector.tensor_copy(betab_sb[:, c], vb[:, 256:260])

            # dpfp: g[j] = f[j] * f[j-1 mod S]  (token-major, 4 heads batched)
            qd = work.tile([P, HG, S], bf16, tag="qd")
            kd = work.tile([P, HG, S], bf16, tag="kd")
            for f, g in ((fq, qd), (fk, kd)):
                nc.vector.tensor_tensor(
                    g[:, :, 1:S], f[:, :, 1:S], f[:, :, 0 : S - 1], OP.mult
                )
                nc.vector.tensor_tensor(
                    g[:, :, 0:1], f[:, :, 0:1], f[:, :, S - 1 : S], OP.mult
                )

            # transpose dpfp tiles -> feature-major
            pt_k = psT.tile([P, 512], bf16, tag="bigb")
            for h in range(HG):
                nc.tensor.matmul(pt_k[:, h * P : (h + 1) * P], kd[:, h], ident,
                                 is_transpose=True, start=(h == 0), stop=(h == HG - 1))
            kT = work.tile([P, HG * P], bf16, tag="kT")
            nc.vector.tensor_copy(kT, pt_k)

            pt_q = psT.tile([P, 512], bf16, tag="bigb")
            for h in range(HG):
                nc.tensor.matmul(pt_q[:, h * P : (h + 1) * P], qd[:, h], ident,
                                 is_transpose=True, start=(h == 0), stop=(h == HG - 1))
            nc.scalar.copy(out=qT_sb[:, c], in_=pt_q)

            # vo_raw | ksum  =  dpfp(k)^T.T @ [W_h | ones]
            pvo = psB.tile([P, HG, 65], f32, tag="small")
            for h in range(HG):
                nc.tensor.matmul(
                    pvo[:, h], kT[:, h * P : (h + 1) * P], wh1[:, h],
                    start=(h == 0), stop=(h == HG - 1),
                )

            vo_sb = work.tile([P, HG, 65], f32, tag="vo_sb")
            nc.vector.tensor_copy(vo_sb, pvo)
            invk = coef.tile([P, HG], f32, tag="invk")
            nc.vector.reciprocal(invk, vo_sb[:, :, 64])
            c1 = coef.tile([P, HG], f32, tag="c1")
            nc.gpsimd.tensor_tensor(c1, betab_sb[:, c], invk, OP.mult)
            c2 = coef.tile([P, HG], f32, tag="c2")
            nc.gpsimd.tensor_tensor(c2, c1, invk, OP.mult)

            # dv' = c1*v - c2*vo_raw   (per token+head scales)
            dv = work.tile([P, HG, D], bf16, tag="dv")
            t1 = work.tile([P, HG, D], bf16, tag="t1")
            nc.vector.tensor_tensor(
                t1, vo_sb[:, :, 0:D], c2[:, :, None].to_broadcast((P, HG, D)), OP.mult
            )
            nc.vector.tensor_tensor(
                dv,
                v_sb[:, c].rearrange("p (h d) -> p h d", h=HG),
                c1[:, :, None].to_broadcast((P, HG, D)),
                OP.mult,
            )
            nc.vector.tensor_tensor(dv, dv, t1, OP.subtract)

            # W update accumulation: pW[h] += kd_h^T @ dv_h
            for h in range(HG):
                nc.tensor.matmul(
                    pW[:, h], kd[:, h], dv[:, h],
                    start=(c == 0 and h == 0), stop=(c == NCH - 1 and h == HG - 1),
                )

        # =============== phase 1.5: W_new ===============
        wn = const.tile([P, HG, D], f32)
        nc.vector.tensor_tensor(wn, pW, wf, OP.add)
        nc.sync.dma_start(out=wnew_d.rearrange("h p d -> p h d"), in_=wn)
        wnb = const.tile([P, HG, 65], bf16)  # [W_new | ones] for readout+qsum
        nc.scalar.copy(out=wnb[:, :, 0:D], in_=wn)
        nc.vector.memset(wnb[:, :, 64:65], 1.0)

        # =============== phase 2: readout + projection ===============
        for c in range(NCH):
            po = psB.tile([P, HG, 65], f32, tag="small")
            for h in range(HG):
                nc.tensor.matmul(
                    po[:, h], qT_sb[:, c, h * P : (h + 1) * P], wnb[:, h],
                    start=(h == 0), stop=(h == HG - 1),
                )
            po_sb = work.tile([P, HG, 65], f32, tag="po_sb")
            nc.vector.tensor_copy(po_sb, po)
            invq = coef.tile([P, HG], f32, tag="invq")
            nc.vector.reciprocal(invq, po_sb[:, :, 64])
            oh = work.tile([P, HG, D], bf16, tag="oh")
            nc.vector.tensor_tensor(
                oh, po_sb[:, :, 0:D], invq[:, :, None].to_broadcast((P, HG, D)), OP.mult
            )
            oh2 = oh.rearrange("p h d -> p (h d)")
            poT = psT.tile([P, 512], bf16, tag="bigb")
            for j in range(2):
                nc.tensor.matmul(
                    poT[:, j * P : (j + 1) * P], oh2[:, j * P : (j + 1) * P], ident,
                    is_transpose=True, start=(j == 0), stop=(j == 1),
                )
            ohT = work.tile([P, 256], bf16, tag="ohT")
            nc.scalar.copy(out=ohT, in_=poT[:, 0:256])

            pp = psA.tile([P, 512], f32, tag="big")
            for j in range(2):
                nc.tensor.matmul(
                    pp, ohT[:, j * P : (j + 1) * P], wout[:, j],
                    start=(j == 0), stop=(j == 1),
                )
            ot = work.tile([P, DIM], f32, tag="ot")
            if c % 2 == 0:
                nc.vector.tensor_copy(ot, pp)
            else:
                nc.scalar.copy(out=ot, in_=pp)
            nc.sync.dma_start(out=out_d[c * P : (c + 1) * P, :], in_=ot)
    nc.compile()
    return nc


def _prep_inputs(x, W, wqkv, wbeta, bbeta, wout):
    """Build the 8 per-core input maps (host-side shard + cast)."""
    in_maps = []
    for core in range(8):
        b = core // 2
        h0 = (core % 2) * HG
        cq = slice(h0 * D, (h0 + HG) * D)
        bbrow = np.zeros((1, 260), BF16)
        bbrow[0, 256:260] = bbeta[h0 : h0 + HG].astype(BF16)
        wq = wqkv[:, 0:512][:, cq]
        wk = wqkv[:, 512:1024][:, cq]
        wv = wqkv[:, 1024:1536][:, cq]
        wqkvb = np.concatenate(
            [wq, wk, wv, wbeta[:, h0 : h0 + HG]], axis=1
        ).astype(BF16)
        Wsl = W[b, h0 : h0 + HG]  # [4,128,64]
        wh1 = np.concatenate(
            [Wsl, np.ones((HG, S, 1), np.float32)], axis=2
        ).astype(BF16)
        in_maps.append(
            {
                "xt": np.ascontiguousarray(x[b].T).astype(BF16),
                "wqkvb": np.ascontiguousarray(wqkvb),
                "wh1": np.ascontiguousarray(wh1),
                "wf": np.ascontiguousarray(Wsl.astype(np.float32)),
                "wout_s": np.ascontiguousarray(
                    (wout[cq, :] * SCALE).astype(BF16)
                ),
                "bbrow": bbrow,
            }
        )
    return in_maps


def kernel(x, W, wqkv, wbeta, bbeta, wout, bout, _trace=False):
    from concourse import bass_utils

    if "nc" not in _CACHE:
        _CACHE["nc"] = _build_nc()
    nc = _CACHE["nc"]
    in_maps = _prep_inputs(
        np.asarray(x, np.float32),
        np.asarray(W, np.float32),
        np.asarray(wqkv, np.float32),
        np.asarray(wbeta, np.float32),
        np.asarray(bbeta, np.float32),
        np.asarray(wout, np.float32),
    )
    try:
        res = bass_utils.run_bass_kernel_spmd(
            nc, in_maps, core_ids=list(range(8)), trace=_trace
        )
    except ModuleNotFoundError:
        res = bass_utils.run_bass_kernel_spmd(
            nc, in_maps, core_ids=list(range(8)), trace=False
        )
    bout = np.asarray(bout, np.float32)
    out = np.empty((B, N, DIM), np.float32)
    W_new = np.empty((B, H, S, D), np.float32)
    for core in range(8):
        b = core // 2
        h0 = (core % 2) * HG
        r = res.results[core]
        if core % 2 == 0:
            out[b] = r["out_part"]
        else:
            out[b] += r["out_part"]
        W_new[b, h0 : h0 + HG] = r["w_new"]
    out += bout
    if _trace:
        return (out, W_new), res
    return out, W_new
